# revision 1
# baseline (speedup 1.0000x reference)
"""CantorAttention Trainium2 kernel.

Problem (hardcoded): B=2, S=2048, DIM=512, H=8 heads, D=64, K=64 routes.
  qkv = x @ w_qkv + b_qkv ; per-head sparse attention over routes[q, :] ;
  out = attn_out @ w_out + b_out.

Strategy (8 cores): shard batch x head-pairs. Core i handles batch i//4 and
heads (2*(i%4), 2*(i%4)+1). Routes are shared across batch/heads, so the
sparse attention is run DENSE on the TensorEngine with a host-precomputed
multiplicative count-mask C^T[k, q] = #{j : routes[q, j] == k}:

  P[k, q]  = C^T[k, q] * exp(scale * (K q_vec . k_vec))       (0 off-route)
  out_h    = (V^T_aug @ P) / denom,  denom from an appended ones-column
  partial  = concat_h(out_h) @ w_out[head rows]               (per core)

Host gathers: final[b] = sum of the 4 partials of batch b + b_out.
Exactly reproduces softmax over the 64 routed scores (duplicates included
via the count mask).

Everything on PE is bf16 with fp32 PSUM accumulation; exp on ScalarE;
mask-multiply on VectorE (bf16 2x mode); transposed layouts throughout so
no on-chip transposes are needed except V (one PE transpose per key tile).
"""

import numpy as np
import ml_dtypes

import concourse.bass as bass
import concourse.bacc as bacc
import concourse.mybir as mybir
import concourse.tile as tile
from concourse.bass_utils import run_bass_kernel_spmd
from concourse.masks import make_identity

BF16 = mybir.dt.bfloat16
F32 = mybir.dt.float32
NPBF16 = ml_dtypes.bfloat16

B = 2
S = 2048
DIM = 512
H = 8
D = 64
KR = 64
SCALE = 0.125

P = 128
NKT = S // P      # 16 key tiles
QC = 512          # query chunk (psum bank width)
NQC = S // QC     # 4 query chunks
NC4 = DIM // P    # 4 contraction chunks

_CACHE = {}


def build_nc():
    if "nc" in _CACHE:
        return _CACHE["nc"]
    nc = bacc.Bacc(
        "TRN2",
        target_bir_lowering=False,
        debug=False,
        num_devices=8,
    )

    xt_d = nc.dram_tensor("xt", [P, NC4 * S], BF16, kind="ExternalInput").ap()
    wq_d = nc.dram_tensor("wq", [P, NC4 * P], BF16, kind="ExternalInput").ap()
    wk_d = nc.dram_tensor("wk", [P, NC4 * P], BF16, kind="ExternalInput").ap()
    wv_d = nc.dram_tensor("wv", [P, NC4 * P], BF16, kind="ExternalInput").ap()
    bq_d = nc.dram_tensor("bq", [P, 1], F32, kind="ExternalInput").ap()
    bk_d = nc.dram_tensor("bk", [P, 1], F32, kind="ExternalInput").ap()
    bv_d = nc.dram_tensor("bv", [P, 1], F32, kind="ExternalInput").ap()
    ct_d = nc.dram_tensor("ct", [P, NKT * S], BF16, kind="ExternalInput").ap()
    wo_d = nc.dram_tensor("wo", [P, DIM], BF16, kind="ExternalInput").ap()
    out_d = nc.dram_tensor("out", [S, DIM], F32, kind="ExternalOutput").ap()

    with tile.TileContext(nc) as tc:
        with tc.tile_pool(name="persist", bufs=1) as pp:
            ident = pp.tile([P, P], BF16, tag="ident")
            make_identity(nc, ident[:])

            xt_big = pp.tile([P, NC4 * S], BF16, tag="xtb", name="xt_big")
            nc.sync.dma_start(out=xt_big[:], in_=xt_d[:, :])
            xt_sb = [xt_big[:, c * S:(c + 1) * S] for c in range(NC4)]

            w_sb = {}
            for name, wd in (("q", wq_d), ("k", wk_d), ("v", wv_d)):
                wt = pp.tile([P, NC4 * P], BF16, tag=f"w{name}b", name=f"w{name}_big")
                nc.sync.dma_start(out=wt[:], in_=wd[:, :])
                for c in range(NC4):
                    w_sb[(name, c)] = wt[:, c * P:(c + 1) * P]
            b_sb = {}
            for name, bd in (("q", bq_d), ("k", bk_d), ("v", bv_d)):
                t = pp.tile([P, 1], F32, tag=f"b{name}", name=f"b{name}_sb")
                nc.sync.dma_start(out=t[:], in_=bd[:, :])
                b_sb[name] = t

            wo_sb = pp.tile([P, DIM], BF16, tag="wo")
            nc.sync.dma_start(out=wo_sb[:], in_=wo_d[:, :])
            sel_sb = {}
            for h in range(2):
                t = pp.tile([P, P], F32, tag=f"sel{h}", name=f"sel{h}")
                nc.vector.memset(t[:], 0.0)
                nc.vector.memset(t[0:1, h * D:(h + 1) * D], 1.0)
                sel_sb[h] = t

            ct_big = pp.tile([P, NKT * S], BF16, tag="ctb", name="ct_big")
            nc.sync.dma_start(out=ct_big[:], in_=ct_d[:, :])
            ct_sb = [ct_big[:, kt * S:(kt + 1) * S] for kt in range(NKT)]

            # v^T stacked (2 heads); q^T/k^T per-head, rows 64-127 zero-padded
            # so every main-loop matmul is a full [128,128] stationary operand.
            qkvt = {}
            qkvt["v"] = pp.tile([P, S], BF16, tag="vt", name="vt")
            for name in ("q", "k"):
                for h in range(2):
                    t = pp.tile([P, S], BF16, tag=f"{name}t{h}", name=f"{name}t{h}")
                    nc.vector.memset(t[D:P, :], 0.0)
                    qkvt[(name, h)] = t

            # Phase 1: QKV^T = W^T @ X^T (+bias), bf16.
            with tc.tile_pool(name="psum_pre", bufs=4, space="PSUM") as pre:
                for name in ("k", "q", "v"):
                    for qc in range(NQC):
                        ps = pre.tile([P, QC], F32, tag="qkvps", name="qkvps")
                        for c in range(NC4):
                            nc.tensor.matmul(
                                ps[:],
                                lhsT=w_sb[(name, c)],
                                rhs=xt_sb[c][:, qc * QC:(qc + 1) * QC],
                                start=(c == 0),
                                stop=(c == NC4 - 1),
                            )
                        if name == "v":
                            nc.vector.tensor_tensor(
                                out=qkvt["v"][:, qc * QC:(qc + 1) * QC],
                                in0=ps[:],
                                in1=b_sb["v"][:].to_broadcast([P, QC]),
                                op=mybir.AluOpType.add,
                            )
                        else:
                            for h in range(2):
                                hd = h * D
                                nc.vector.tensor_tensor(
                                    out=qkvt[(name, h)][0:D, qc * QC:(qc + 1) * QC],
                                    in0=ps[hd:hd + D, :],
                                    in1=b_sb[name][hd:hd + D, :].to_broadcast([D, QC]),
                                    op=mybir.AluOpType.add,
                                )

                # Phase 1b: V tiles in [key, d] layout with ones column.
                v_sb = {}
                for h in range(2):
                    for kt in range(NKT):
                        v_sb[(h, kt)] = pp.tile([P, P], BF16, tag=f"v{h}_{kt}", name=f"v{h}_{kt}")
                for kt in range(NKT):
                    tp = pre.tile([P, P], BF16, tag="vtps", name="vtps")
                    nc.tensor.transpose(
                        out=tp[:], in_=qkvt["v"][:, kt * P:(kt + 1) * P],
                        identity=ident[:],
                    )
                    for h in range(2):
                        nc.scalar.copy(
                            out=v_sb[(h, kt)][:, 0:D], in_=tp[:, h * D:(h + 1) * D]
                        )
                        nc.vector.memset(v_sb[(h, kt)][:, D:D + 1], 1.0)
                        nc.vector.memset(v_sb[(h, kt)][:, D + 1:P], 0.0)

            ot_sb = pp.tile([P, S], F32, tag="ot")
            den_sb = {}
            for h in range(2):
                den_sb[h] = pp.tile([P, S], F32, tag=f"den{h}", name=f"den{h}")
                nc.vector.memset(den_sb[h][D:P, :], 0.0)
                nc.vector.memset(den_sb[h][0:D, :], 0.0)
            r2r_sb = pp.tile([P, S], F32, tag="r2r")
            on_sb = pp.tile([P, S], BF16, tag="on")

            # Phase 2: dense masked attention, one head at a time.
            QH = 1024
            for h in range(2):
                hd = h * D
                with tc.tile_pool(name=f"psum_s{h}", bufs=2, space="PSUM") as sp, \
                     tc.tile_pool(name=f"psum_ot{h}", bufs=1, space="PSUM") as op, \
                     tc.tile_pool(name=f"pwork{h}", bufs=6) as pw:
                    ot_ps = op.tile([P, S], F32, tag="otps", name="otps")
                    for kt in range(NKT):
                        for q2 in range(S // QH):
                            s_ps = sp.tile([P, QH], F32, tag="s", name="s_ps")
                            for half in range(QH // QC):
                                off = q2 * QH + half * QC
                                nc.tensor.matmul(
                                    s_ps[:, half * QC:(half + 1) * QC],
                                    lhsT=qkvt[("k", h)][:, kt * P:(kt + 1) * P],
                                    rhs=qkvt[("q", h)][:, off:off + QC],
                                    start=True,
                                    stop=True,
                                )
                            p_sb = pw.tile([P, QH], BF16, tag="p", name="p_sb")
                            nc.scalar.activation(
                                p_sb[:], s_ps[:], mybir.ActivationFunctionType.Exp
                            )
                            pm_sb = pw.tile([P, QH], BF16, tag="pm", name="pm_sb")
                            nc.vector.tensor_tensor(
                                out=pm_sb[:],
                                in0=p_sb[:],
                                in1=ct_sb[kt][:, q2 * QH:(q2 + 1) * QH],
                                op=mybir.AluOpType.mult,
                            )
                            for half in range(QH // QC):
                                off = q2 * QH + half * QC
                                nc.tensor.matmul(
                                    ot_ps[:, off:off + QC],
                                    lhsT=v_sb[(h, kt)][:],
                                    rhs=pm_sb[:, half * QC:(half + 1) * QC],
                                    start=(kt == 0),
                                    stop=(kt == NKT - 1),
                                )
                    nc.scalar.copy(out=ot_sb[hd:hd + D, :], in_=ot_ps[0:D, :])
                    nc.vector.tensor_copy(out=den_sb[h][0:1, :], in_=ot_ps[D:D + 1, :])

            # Phase 3: normalize, project, store (pipelined per 512-chunk).
            with tc.tile_pool(name="psum_r2", bufs=2, space="PSUM") as rp, \
                 tc.tile_pool(name="psum_fin", bufs=3, space="PSUM") as fp, \
                 tc.tile_pool(name="fin_sb", bufs=4) as fsb:
                r2_list = []
                for qc in range(NQC):
                    qs = slice(qc * QC, (qc + 1) * QC)
                    r2_ps = rp.tile([P, QC], F32, tag="r2", name="r2_ps", bufs=4)
                    for h in range(2):
                        nc.tensor.matmul(
                            r2_ps[:],
                            lhsT=sel_sb[h][:],
                            rhs=den_sb[h][:, qs],
                            start=(h == 0),
                            stop=(h == 1),
                        )
                    r2_list.append(r2_ps)
                for qc in range(NQC):
                    qs = slice(qc * QC, (qc + 1) * QC)
                    nc.vector.reciprocal_approx_fast(out=r2r_sb[:, qs], in_=r2_list[qc][:])
                    nc.vector.tensor_tensor(
                        out=on_sb[:, qs], in0=ot_sb[:, qs], in1=r2r_sb[:, qs],
                        op=mybir.AluOpType.mult,
                    )
                    for qt in range(qc * NC4, (qc + 1) * NC4):
                        pr = fp.tile([P, DIM], F32, tag="pr", name="pr_ps")
                        nc.tensor.matmul(
                            pr[:],
                            lhsT=on_sb[:, qt * P:(qt + 1) * P],
                            rhs=wo_sb[:],
                            start=True,
                            stop=True,
                        )
                        o_sb = fsb.tile([P, DIM], F32, tag="osb", name="o_sb")
                        nc.scalar.copy(out=o_sb[:], in_=pr[:])
                        nc.sync.dma_start(
                            out=out_d[qt * P:(qt + 1) * P, :], in_=o_sb[:]
                        )

    nc.compile()
    _CACHE["nc"] = nc
    return nc


def make_in_maps(x, routes, w_qkv, b_qkv, w_out):
    x = np.asarray(x, np.float32)
    routes = np.asarray(routes)
    w_qkv = np.asarray(w_qkv, np.float32)
    b_qkv = np.asarray(b_qkv, np.float32)
    w_out = np.asarray(w_out, np.float32)

    C = np.zeros((S, S), np.float32)
    np.add.at(C, (np.arange(S)[:, None], routes), 1.0)

    def pack(a):
        # [n*128, X] -> [128, n*X]
        n = a.shape[0] // P
        return np.ascontiguousarray(
            a.reshape(n, P, a.shape[1]).transpose(1, 0, 2).reshape(P, -1))

    xt = [pack(np.ascontiguousarray(x[b].T)).astype(NPBF16) for b in range(B)]
    ctp = pack(np.ascontiguousarray(C.T)).astype(NPBF16)

    in_maps = []
    for core in range(8):
        b = core // 4
        hp = core % 4
        col = hp * P
        wq = pack(w_qkv[:, col:col + P] * SCALE).astype(NPBF16)
        wk = pack(w_qkv[:, DIM + col:DIM + col + P]).astype(NPBF16)
        wv = pack(w_qkv[:, 2 * DIM + col:2 * DIM + col + P]).astype(NPBF16)
        bq = (b_qkv[col:col + P] * SCALE).astype(np.float32).reshape(P, 1)
        bk = b_qkv[DIM + col:DIM + col + P].astype(np.float32).reshape(P, 1)
        bv = b_qkv[2 * DIM + col:2 * DIM + col + P].astype(np.float32).reshape(P, 1)
        wo = np.ascontiguousarray(w_out[col:col + P, :]).astype(NPBF16)
        in_maps.append(dict(
            xt=xt[b], wq=wq, wk=wk, wv=wv, bq=bq, bk=bk, bv=bv,
            ct=ctp, wo=wo,
        ))
    return in_maps


def run(inputs, trace=False, trace_cores=None):
    nc = build_nc()
    in_maps = make_in_maps(
        inputs["x"], inputs["routes"], inputs["w_qkv"], inputs["b_qkv"],
        inputs["w_out"],
    )
    res = run_bass_kernel_spmd(
        nc, in_maps, list(range(8)), trace=trace, trace_cores=trace_cores,
    )
    b_out = np.asarray(inputs["b_out"], np.float32)
    final = np.zeros((B, S, DIM), np.float32)
    for core in range(8):
        final[core // 4] += res.results[core]["out"]
    final += b_out[None, None, :]
    return final, res


def kernel(**inputs):
    final, _ = run(inputs, trace=False)
    return final



# revision 10
# speedup vs baseline: 1.7017x; 1.7017x over previous
"""CantorAttention Trainium2 kernel — block-sparse routed attention.

Problem (hardcoded): B=2, S=2048, DIM=512, H=8 heads, D=64, K=64 routes.
  qkv = x @ w_qkv + b_qkv ; per-head softmax attention over routes[q, :] ;
  out = attn_out @ w_out + b_out.

Strategy (8 cores): shard batch x head-pairs. Core i handles batch i//4 and
heads (2*(i%4), 2*(i%4)+1).

Sparsity exploit: routes are distinct per query (binary mask).  A single
token permutation (iterated sort by mean routed-neighbour index — derived
from the routes alone) makes the [S, S] route mask block-sparse: each
256-query block touches only a few 128-key tiles.  The host builds that
schedule and compiles a kernel specialized to it; attention runs dense only
on the touched (key-tile, query-block) pairs with an additive {0, -200}
mask folded into the score PSUM via an identity matmul.

Algebraic simplifications:
  - K-bias dropped: (q+bq).(k+bk) differs from (q+bq).k by a per-query
    constant -> cancels in softmax.
  - V-bias folded into the output bias on the host (softmax weights sum
    to 1), so V = x @ wv with no bias and the host adds
    b_out + b_qkv[2*DIM:] @ w_out once.
  - Denominators ride along in the AV matmul via a ones column appended to
    V (output row 64), then one reciprocal + a ones-row broadcast matmul
    replicates 1/den across the head's 64 partitions for normalization.

Both heads of a core run concurrently in the QK matmuls via PE row tiling
(contraction 64 each, tile_position rows 0-63 / 64-127).
"""

import numpy as np
import ml_dtypes

import concourse.bass as bass
import concourse.bacc as bacc
import concourse.mybir as mybir
import concourse.tile as tile
from concourse.bass_utils import run_bass_kernel_spmd
from concourse.masks import make_identity

BF16 = mybir.dt.bfloat16
F32 = mybir.dt.float32
NPBF16 = ml_dtypes.bfloat16

B = 2
S = 2048
DIM = 512
H = 8
D = 64
KR = 64
SCALE = 0.125

P = 128
NC4 = DIM // P    # 4 contraction chunks
QB = 256          # query block
NQB = S // QB     # 8 query blocks
NKT = S // P      # 16 key tiles
VW = D + 1        # v tile width incl ones column
MASKNEG = -200.0

_CACHE = {}
DBG = set()  # debug: "nonorm", "expbank", "norowtile"


def _token_order(routes):
    """Permutation clustering tokens so each query block touches few key
    tiles.  Iterated argsort by mean routed-neighbour position; generic
    (no Cantor assumption) and a no-op perf-wise for random routes."""
    n = routes.shape[0]
    order = np.argsort(routes.mean(axis=1), kind="stable")
    for _ in range(3):
        inv = np.empty(n, np.int64)
        inv[order] = np.arange(n)
        m = inv[routes].mean(axis=1)
        order = order[np.argsort(m[order], kind="stable")]
    return order


def _schedule(routes):
    routes = np.asarray(routes, np.int64)
    order = _token_order(routes)
    inv = np.empty(S, np.int64)
    inv[order] = np.arange(S)
    rk = inv[routes][order]          # [S, K] both sides permuted
    tiles = rk // P
    sched = tuple(
        tuple(sorted(set(tiles[qb * QB:(qb + 1) * QB].ravel().tolist())))
        for qb in range(NQB)
    )
    return order, rk, sched


def build_nc(sched):
    key = (sched, tuple(sorted(DBG)))
    if key in _CACHE:
        return _CACHE[key]
    npair = sum(len(k) for k in sched)
    nc = bacc.Bacc(
        "TRN2",
        target_bir_lowering=False,
        debug=False,
        num_devices=8,
    )

    xt_d = nc.dram_tensor("xt", [P, NC4 * S], BF16, kind="ExternalInput").ap()
    wq_d = nc.dram_tensor("wq", [P, NC4 * P], BF16, kind="ExternalInput").ap()
    wk_d = nc.dram_tensor("wk", [P, NC4 * P], BF16, kind="ExternalInput").ap()
    wv_d = nc.dram_tensor("wv", [P, NC4 * P], BF16, kind="ExternalInput").ap()
    bq_d = nc.dram_tensor("bq", [P, 1], F32, kind="ExternalInput").ap()
    msk_d = nc.dram_tensor("msk", [P, npair * QB], BF16, kind="ExternalInput").ap()
    wo_d = nc.dram_tensor("wo", [P, DIM], BF16, kind="ExternalInput").ap()
    out_d = nc.dram_tensor("out", [S, DIM], F32, kind="ExternalOutput").ap()
    if "dump" in DBG:
        dq_d = nc.dram_tensor("dq", [P, S], F32, kind="ExternalOutput").ap()
        dk_d = nc.dram_tensor("dk", [P, S], F32, kind="ExternalOutput").ap()
        dv0_d = nc.dram_tensor("dv0", [P, NKT * VW], F32, kind="ExternalOutput").ap()
        dv1_d = nc.dram_tensor("dv1", [P, NKT * VW], F32, kind="ExternalOutput").ap()
        don_d = nc.dram_tensor("don", [P, S], F32, kind="ExternalOutput").ap()
        do_d = nc.dram_tensor("do_", [P, NQB * 2 * QB], F32, kind="ExternalOutput").ap()
        dpm_d = nc.dram_tensor("dpm", [P, NQB * 4 * QB], F32, kind="ExternalOutput").ap()

    qb_off = []
    off = 0
    for kts in sched:
        qb_off.append(off)
        off += len(kts) * QB

    with tile.TileContext(nc) as tc:
        with tc.tile_pool(name="persist", bufs=1) as pp:
            ident = pp.tile([P, P], BF16, tag="ident")
            make_identity(nc, ident[:])
            onesr = pp.tile([1, P], F32, tag="onesr")
            nc.vector.memset(onesr[:], 1.0)

            xt_sb = pp.tile([P, NC4 * S], BF16, tag="xt", name="xt_sb")
            for c in range(NC4):
                nc.sync.dma_start(
                    out=xt_sb[:, c * S:(c + 1) * S], in_=xt_d[:, c * S:(c + 1) * S]
                )
            w_sb = {}
            for name, wd in (("q", wq_d), ("k", wk_d), ("v", wv_d)):
                t = pp.tile([P, NC4 * P], BF16, tag=f"w{name}", name=f"w{name}_sb")
                nc.sync.dma_start(out=t[:], in_=wd[:, :])
                w_sb[name] = t
            bq_sb = pp.tile([P, 1], F32, tag="bq")
            nc.sync.dma_start(out=bq_sb[:], in_=bq_d[:, :])
            wo_sb = pp.tile([P, DIM], BF16, tag="wo")
            nc.sync.dma_start(out=wo_sb[:], in_=wo_d[:, :])
            msk_sb = pp.tile([P, npair * QB], BF16, tag="msk", name="msk_sb")
            for qb in range(NQB):
                w = len(sched[qb]) * QB
                o0 = qb_off[qb]
                nc.sync.dma_start(
                    out=msk_sb[:, o0:o0 + w], in_=msk_d[:, o0:o0 + w]
                )

            qT = pp.tile([P, S], BF16, tag="qT")
            kT = pp.tile([P, S], BF16, tag="kT")
            v0 = pp.tile([P, NKT * VW], BF16, tag="v0")
            v1 = pp.tile([P, NKT * VW], BF16, tag="v1")
            nc.vector.memset(v0[:], 1.0)
            nc.vector.memset(v1[:], 1.0)
            on_sb = pp.tile([P, S], BF16, tag="on")

            # ---- Phase A: projections ----
            QC = 512
            with tc.tile_pool(name="pa", bufs=3, space="PSUM") as pa:
                for qc in range(NC4):          # kT = Wk^T X^T  (no bias)
                    ps = pa.tile([P, QC], F32, tag="qkps", name="kps")
                    for c in range(NC4):
                        nc.tensor.matmul(
                            ps[:],
                            lhsT=w_sb["k"][:, c * P:(c + 1) * P],
                            rhs=xt_sb[:, c * S + qc * QC: c * S + qc * QC + QC],
                            start=(c == 0),
                            stop=(c == NC4 - 1),
                        )
                    if qc % 2 == 0:
                        nc.scalar.copy(out=kT[:, qc * QC:(qc + 1) * QC], in_=ps[:])
                    else:
                        nc.vector.tensor_copy(
                            out=kT[:, qc * QC:(qc + 1) * QC], in_=ps[:]
                        )
                for qc in range(NC4):          # qT = Wq'^T X^T + bq'  (scaled)
                    ps = pa.tile([P, QC], F32, tag="qkps", name="qps")
                    for c in range(NC4):
                        nc.tensor.matmul(
                            ps[:],
                            lhsT=w_sb["q"][:, c * P:(c + 1) * P],
                            rhs=xt_sb[:, c * S + qc * QC: c * S + qc * QC + QC],
                            start=(c == 0),
                            stop=(c == NC4 - 1),
                        )
                    nc.vector.tensor_tensor(
                        out=qT[:, qc * QC:(qc + 1) * QC],
                        in0=ps[:],
                        in1=bq_sb[:].to_broadcast([P, QC]),
                        op=mybir.AluOpType.add,
                    )
                for kt in range(NKT):          # V direct form [tokens, dims]
                    vps = pa.tile([P, P], F32, tag="vps", name="vps")
                    for c in range(NC4):
                        nc.tensor.matmul(
                            vps[:],
                            lhsT=xt_sb[:, c * S + kt * P: c * S + kt * P + P],
                            rhs=w_sb["v"][:, c * P:(c + 1) * P],
                            start=(c == 0),
                            stop=(c == NC4 - 1),
                        )
                    d0 = kt * VW
                    if kt % 2 == 0:
                        nc.vector.tensor_copy(out=v0[:, d0:d0 + D], in_=vps[:, 0:D])
                        nc.scalar.copy(out=v1[:, d0:d0 + D], in_=vps[:, D:2 * D])
                    else:
                        nc.scalar.copy(out=v0[:, d0:d0 + D], in_=vps[:, 0:D])
                        nc.vector.tensor_copy(out=v1[:, d0:d0 + D], in_=vps[:, D:2 * D])

            # ---- Phase B: block-sparse attention + normalize + project ----
            with tc.tile_pool(name="ps_s", bufs=2, space="PSUM") as sp, \
                 tc.tile_pool(name="ps_o", bufs=2, space="PSUM") as opool, \
                 tc.tile_pool(name="ps_r", bufs=1, space="PSUM") as rp, \
                 tc.tile_pool(name="ps_pr", bufs=1, space="PSUM") as prp, \
                 tc.tile_pool(name="pb", bufs=3) as pb, \
                 tc.tile_pool(name="fin", bufs=3) as fsb:
                for qb in range(NQB):
                    kts = sched[qb]
                    nkt = len(kts)
                    qs = slice(qb * QB, (qb + 1) * QB)
                    o = opool.tile([P, 2 * QB], F32, tag="o", name="o")
                    groups = [kts[i:i + 2] for i in range(0, nkt, 2)]
                    pi = 0
                    for g in groups:
                        # s spans 2 banks: bank A (cols 0:2QB) holds h0
                        # scores for the group's pairs, bank B (2QB:4QB)
                        # holds h1.  Row-group-64 matmuls must not share a
                        # bank with row-group-0 ones (HW crash), so heads
                        # get separate banks; ident mask matmuls (full 128)
                        # may share with either.
                        ng = len(g)
                        s = sp.tile([P, 4 * QB], F32, tag="s", name="s")
                        pmt = pb.tile([P, 4 * QB], BF16, tag="pm", name="pm")
                        for jj, kt in enumerate(g):
                            a0 = jj * QB              # h0 region (bank A)
                            a1 = 2 * QB + jj * QB     # h1 region (bank B)
                            moff = qb_off[qb] + (pi + jj) * QB
                            if "norowtile" in DBG:
                                nc.tensor.matmul(
                                    s[:, a0:a0 + QB],
                                    lhsT=kT[:, kt * P:(kt + 1) * P],
                                    rhs=qT[:, qs],
                                    start=(jj == 0), stop=False,
                                )
                                nc.tensor.matmul(
                                    s[:, a1:a1 + QB],
                                    lhsT=kT[:, kt * P:(kt + 1) * P],
                                    rhs=qT[:, qs],
                                    start=(jj == 0), stop=False,
                                )
                            else:
                                nc.tensor.matmul(
                                    s[:, a0:a0 + QB],
                                    lhsT=kT[0:D, kt * P:(kt + 1) * P],
                                    rhs=qT[0:D, qs],
                                    start=(jj == 0), stop=False,
                                )
                                nc.tensor.matmul(
                                    s[:, a1:a1 + QB],
                                    lhsT=kT[D:P, kt * P:(kt + 1) * P],
                                    rhs=qT[D:P, qs],
                                    start=(jj == 0), stop=False,
                                )
                            nc.tensor.matmul(
                                s[:, a0:a0 + QB],
                                lhsT=ident[:],
                                rhs=msk_sb[:, moff:moff + QB],
                                start=False, stop=(jj == ng - 1),
                            )
                            nc.tensor.matmul(
                                s[:, a1:a1 + QB],
                                lhsT=ident[:],
                                rhs=msk_sb[:, moff:moff + QB],
                                start=False, stop=(jj == ng - 1),
                            )
                        if ng == 2:
                            nc.scalar.activation(
                                pmt[:], s[:], mybir.ActivationFunctionType.Exp
                            )
                        else:
                            nc.scalar.activation(
                                pmt[:, 0:QB], s[:, 0:QB],
                                mybir.ActivationFunctionType.Exp
                            )
                            nc.scalar.activation(
                                pmt[:, 2 * QB:3 * QB], s[:, 2 * QB:3 * QB],
                                mybir.ActivationFunctionType.Exp
                            )
                        for jj, kt in enumerate(g):
                            j = pi + jj
                            a0 = jj * QB
                            a1 = 2 * QB + jj * QB
                            nc.tensor.matmul(
                                o[0:VW, 0:QB],
                                lhsT=v0[:, kt * VW:(kt + 1) * VW],
                                rhs=pmt[:, a0:a0 + QB],
                                start=(j == 0), stop=False,
                            )
                            nc.tensor.matmul(
                                o[0:VW, QB:2 * QB],
                                lhsT=v1[:, kt * VW:(kt + 1) * VW],
                                rhs=pmt[:, a1:a1 + QB],
                                start=False, stop=(j == nkt - 1),
                            )
                        pi += len(g)
                        if "dump" in DBG and pi == len(g):
                            dt_ = pb.tile([P, 4 * QB], F32, tag="dmp", name="dmp")
                            nc.scalar.copy(out=dt_[:], in_=pmt[:])
                            nc.sync.dma_start(
                                out=dpm_d[:, qb * 4 * QB: (qb * 4 + 4) * QB],
                                in_=dt_[:])

                    if "dump" in DBG:
                        dt2 = pb.tile([P, 2 * QB], F32, tag="dmp2", name="dmp2")
                        nc.vector.tensor_copy(out=dt2[:], in_=o[:])
                        nc.sync.dma_start(
                            out=do_d[:, qb * 2 * QB:(qb + 1) * 2 * QB], in_=dt2[:])
                    if "nonorm" in DBG:
                        nc.vector.tensor_copy(out=on_sb[0:D, qs], in_=o[0:D, 0:QB])
                        nc.vector.tensor_copy(out=on_sb[D:P, qs], in_=o[0:D, QB:2 * QB])
                    else:
                        den_sb = pb.tile([1, 2 * QB], F32, tag="den", name="den_sb")
                        # custom-DVE recip misreads PSUM at partition base 64;
                        # stage the den row through SBUF first.
                        nc.vector.tensor_copy(out=den_sb[:], in_=o[D:D + 1, :])
                        rd = pb.tile([1, 2 * QB], F32, tag="rd", name="rd")
                        nc.vector.reciprocal_approx_fast(out=rd[:], in_=den_sb[:])
                        rep = rp.tile([P, 2 * QB], F32, tag="rep", name="rep")
                        nc.tensor.matmul(
                            rep[:, 0:QB], lhsT=onesr[:], rhs=rd[0:1, 0:QB],
                            start=True, stop=False,
                        )
                        nc.tensor.matmul(
                            rep[:, QB:2 * QB], lhsT=onesr[:], rhs=rd[0:1, QB:2 * QB],
                            start=False, stop=True,
                        )
                        rep_sb = pb.tile([P, 2 * QB], BF16, tag="repsb", name="rep_sb")
                        nc.vector.tensor_copy(out=rep_sb[:], in_=rep[:])
                        nc.vector.tensor_tensor(
                            out=on_sb[0:D, qs], in0=o[0:D, 0:QB],
                            in1=rep_sb[0:D, 0:QB],
                            op=mybir.AluOpType.mult,
                        )
                        nc.vector.tensor_tensor(
                            out=on_sb[D:P, qs], in0=o[0:D, QB:2 * QB],
                            in1=rep_sb[D:P, QB:2 * QB],
                            op=mybir.AluOpType.mult,
                        )
                    for t in range(QB // P):
                        qt = qb * (QB // P) + t
                        pr = prp.tile([P, DIM], F32, tag="pr", name="pr")
                        nc.tensor.matmul(
                            pr[:],
                            lhsT=on_sb[:, qt * P:(qt + 1) * P],
                            rhs=wo_sb[:],
                            start=True, stop=True,
                        )
                        ob = fsb.tile([P, DIM], F32, tag="ob", name="ob")
                        if qt % 2 == 0:
                            nc.scalar.copy(out=ob[:], in_=pr[:])
                        else:
                            nc.vector.tensor_copy(out=ob[:], in_=pr[:])
                        nc.sync.dma_start(
                            out=out_d[qt * P:(qt + 1) * P, :], in_=ob[:]
                        )

                if "dump" in DBG:
                    for nm, src_t, dst in (("dq", qT, dq_d), ("dk", kT, dk_d),
                                           ("dv0", v0, dv0_d), ("dv1", v1, dv1_d),
                                           ("don", on_sb, don_d)):
                        w = src_t.shape[1]
                        for c0 in range(0, w, 2048):
                            cw = min(2048, w - c0)
                            dt3 = fsb.tile([P, 2048], F32, tag="dmp3", name="dmp3")
                            nc.vector.tensor_copy(out=dt3[:, 0:cw], in_=src_t[:, c0:c0 + cw])
                            nc.sync.dma_start(out=dst[:, c0:c0 + cw], in_=dt3[:, 0:cw])

    nc.compile()
    _CACHE[key] = nc
    return nc


def _pack(a):
    # [n*128, X] -> [128, n*X]
    n = a.shape[0] // P
    return np.ascontiguousarray(
        a.reshape(n, P, a.shape[1]).transpose(1, 0, 2).reshape(P, -1))


def make_in_maps(x, routes, w_qkv, b_qkv, w_out, order, rk, sched):
    x = np.asarray(x, np.float32)
    w_qkv = np.asarray(w_qkv, np.float32)
    b_qkv = np.asarray(b_qkv, np.float32)
    w_out = np.asarray(w_out, np.float32)

    Cm = np.zeros((S, S), bool)
    Cm[np.arange(S)[:, None], rk] = True          # permuted [q', k']
    cols = []
    for qb, kts in enumerate(sched):
        blk = Cm[qb * QB:(qb + 1) * QB]
        for kt in kts:
            sub = blk[:, kt * P:(kt + 1) * P]     # [256 q, 128 k]
            cols.append(np.where(sub.T, 0.0, MASKNEG))
    msk = np.concatenate(cols, axis=1).astype(NPBF16)

    xt = [
        _pack(np.ascontiguousarray(x[b].T[:, order])).astype(NPBF16)
        for b in range(B)
    ]

    in_maps = []
    for core in range(8):
        b = core // 4
        hp = core % 4
        col = hp * P
        wq = _pack(w_qkv[:, col:col + P] * SCALE).astype(NPBF16)
        wk = _pack(w_qkv[:, DIM + col:DIM + col + P]).astype(NPBF16)
        wv = _pack(w_qkv[:, 2 * DIM + col:2 * DIM + col + P]).astype(NPBF16)
        bq = (b_qkv[col:col + P] * SCALE).astype(np.float32).reshape(P, 1)
        wo = np.ascontiguousarray(w_out[col:col + P, :]).astype(NPBF16)
        in_maps.append(dict(
            xt=xt[b], wq=wq, wk=wk, wv=wv, bq=bq, msk=msk, wo=wo,
        ))
    return in_maps


def run(inputs, trace=False, trace_cores=None):
    routes = np.asarray(inputs["routes"])
    order, rk, sched = _schedule(routes)
    nc = build_nc(sched)
    in_maps = make_in_maps(
        inputs["x"], routes, inputs["w_qkv"], inputs["b_qkv"],
        inputs["w_out"], order, rk, sched,
    )
    res = run_bass_kernel_spmd(
        nc, in_maps, list(range(8)), trace=trace, trace_cores=trace_cores,
    )
    b_qkv = np.asarray(inputs["b_qkv"], np.float32)
    w_out = np.asarray(inputs["w_out"], np.float32)
    bias = np.asarray(inputs["b_out"], np.float32) + b_qkv[2 * DIM:] @ w_out
    final = np.zeros((B, S, DIM), np.float32)
    for core in range(8):
        final[core // 4][order] += res.results[core]["out"]
    final += bias[None, None, :]
    return final, res


def kernel(**inputs):
    final, _ = run(inputs, trace=False)
    return final


# revision 11
# speedup vs baseline: 1.8715x; 1.0998x over previous
"""CantorAttention Trainium2 kernel — block-sparse routed attention.

Problem (hardcoded): B=2, S=2048, DIM=512, H=8 heads, D=64, K=64 routes.
  qkv = x @ w_qkv + b_qkv ; per-head softmax attention over routes[q, :] ;
  out = attn_out @ w_out + b_out.

Strategy (8 cores): shard batch x head-pairs. Core i handles batch i//4 and
heads (2*(i%4), 2*(i%4)+1).

Sparsity exploit: routes are distinct per query (binary mask).  A single
token permutation (iterated sort by mean routed-neighbour index — derived
from the routes alone) makes the [S, S] route mask block-sparse: each
256-query block touches only a few 128-key tiles.  The host builds that
schedule and compiles a kernel specialized to it; attention runs dense only
on the touched (key-tile, query-block) pairs with an additive {0, -200}
mask folded into the score PSUM via an identity matmul.

Algebraic simplifications:
  - K-bias dropped: (q+bq).(k+bk) differs from (q+bq).k by a per-query
    constant -> cancels in softmax.
  - V-bias folded into the output bias on the host (softmax weights sum
    to 1), so V = x @ wv with no bias and the host adds
    b_out + b_qkv[2*DIM:] @ w_out once.
  - Denominators ride along in the AV matmul via a ones column appended to
    V (output row 64), then one reciprocal + a ones-row broadcast matmul
    replicates 1/den across the head's 64 partitions for normalization.

Both heads of a core run concurrently in the QK matmuls via PE row tiling
(contraction 64 each, tile_position rows 0-63 / 64-127).
"""

import numpy as np
import ml_dtypes

import concourse.bass as bass
import concourse.bacc as bacc
import concourse.mybir as mybir
import concourse.tile as tile
from concourse.bass_utils import run_bass_kernel_spmd
from concourse.masks import make_identity

BF16 = mybir.dt.bfloat16
F32 = mybir.dt.float32
F8 = mybir.dt.float8e4
NPBF16 = ml_dtypes.bfloat16
NPF8 = ml_dtypes.float8_e4m3

B = 2
S = 2048
DIM = 512
H = 8
D = 64
KR = 64
SCALE = 0.125

P = 128
NC4 = DIM // P    # 4 contraction chunks
QB = 256          # query block
NQB = S // QB     # 8 query blocks
NKT = S // P      # 16 key tiles
VW = D + 1        # v tile width incl ones column
MASKNEG = -192.0  # exact in fp8 e4m3 (TRN max-normal 240); exp(-192) == 0

_CACHE = {}
DBG = set()  # debug: "nonorm", "expbank", "norowtile"


def _token_order(routes):
    """Permutation clustering tokens so each query block touches few key
    tiles.  Iterated argsort by mean routed-neighbour position; generic
    (no Cantor assumption) and a no-op perf-wise for random routes."""
    n = routes.shape[0]
    order = np.argsort(routes.mean(axis=1), kind="stable")
    for _ in range(3):
        inv = np.empty(n, np.int64)
        inv[order] = np.arange(n)
        m = inv[routes].mean(axis=1)
        order = order[np.argsort(m[order], kind="stable")]
    return order


def _schedule(routes):
    routes = np.asarray(routes, np.int64)
    order = _token_order(routes)
    inv = np.empty(S, np.int64)
    inv[order] = np.arange(S)
    rk = inv[routes][order]          # [S, K] both sides permuted
    tiles = rk // P
    sched = tuple(
        tuple(sorted(set(tiles[qb * QB:(qb + 1) * QB].ravel().tolist())))
        for qb in range(NQB)
    )
    return order, rk, sched


def build_nc(sched):
    key = (sched, tuple(sorted(DBG)))
    if key in _CACHE:
        return _CACHE[key]
    npair = sum(len(k) for k in sched)
    nc = bacc.Bacc(
        "TRN2",
        target_bir_lowering=False,
        debug=False,
        num_devices=8,
    )

    xt_d = nc.dram_tensor("xt", [P, NC4 * S], BF16, kind="ExternalInput").ap()
    wq_d = nc.dram_tensor("wq", [P, NC4 * P], BF16, kind="ExternalInput").ap()
    wk_d = nc.dram_tensor("wk", [P, NC4 * P], BF16, kind="ExternalInput").ap()
    wv_d = nc.dram_tensor("wv", [P, NC4 * P], BF16, kind="ExternalInput").ap()
    bq_d = nc.dram_tensor("bq", [P, 1], F32, kind="ExternalInput").ap()
    msk_d = nc.dram_tensor("msk", [P, npair * QB], F8, kind="ExternalInput").ap()
    wo_d = nc.dram_tensor("wo", [P, DIM], BF16, kind="ExternalInput").ap()
    out_d = nc.dram_tensor("out", [S, DIM], BF16, kind="ExternalOutput").ap()
    if "dump" in DBG:
        dq_d = nc.dram_tensor("dq", [P, S], F32, kind="ExternalOutput").ap()
        dk_d = nc.dram_tensor("dk", [P, S], F32, kind="ExternalOutput").ap()
        dv0_d = nc.dram_tensor("dv0", [P, NKT * VW], F32, kind="ExternalOutput").ap()
        dv1_d = nc.dram_tensor("dv1", [P, NKT * VW], F32, kind="ExternalOutput").ap()
        don_d = nc.dram_tensor("don", [P, S], F32, kind="ExternalOutput").ap()
        do_d = nc.dram_tensor("do_", [P, NQB * 2 * QB], F32, kind="ExternalOutput").ap()
        dpm_d = nc.dram_tensor("dpm", [P, NQB * 4 * QB], F32, kind="ExternalOutput").ap()

    qb_off = []
    off = 0
    for kts in sched:
        qb_off.append(off)
        off += len(kts) * QB

    with tile.TileContext(nc) as tc:
        with tc.tile_pool(name="persist", bufs=1) as pp:
            ident = pp.tile([P, P], BF16, tag="ident")
            make_identity(nc, ident[:])
            idf8 = pp.tile([P, P], F8, tag="idf8")
            nc.scalar.copy(out=idf8[:], in_=ident[:])
            onesr = pp.tile([1, P], F32, tag="onesr")
            nc.vector.memset(onesr[:], 1.0)

            # Weights first (small, phase A is blocked on them), then x^T in
            # 16 pieces ordered by phase-A consumption, mask (phase B) last.
            w_sb = {}
            for name, wd in (("k", wk_d), ("q", wq_d), ("v", wv_d)):
                t = pp.tile([P, NC4 * P], BF16, tag=f"w{name}", name=f"w{name}_sb")
                nc.sync.dma_start(out=t[:], in_=wd[:, :])
                w_sb[name] = t
            bq_sb = pp.tile([P, 1], F32, tag="bq")
            nc.sync.dma_start(out=bq_sb[:], in_=bq_d[:, :])
            wo_sb = pp.tile([P, DIM], BF16, tag="wo")
            nc.sync.dma_start(out=wo_sb[:], in_=wo_d[:, :])
            xt_sb = pp.tile([P, NC4 * S], BF16, tag="xt", name="xt_sb")
            for qc in range(NC4):
                for c in range(NC4):
                    o0 = c * S + qc * 512
                    nc.sync.dma_start(
                        out=xt_sb[:, o0:o0 + 512], in_=xt_d[:, o0:o0 + 512]
                    )
            msk_sb = pp.tile([P, npair * QB], F8, tag="msk", name="msk_sb")
            for qb in range(NQB):
                w = len(sched[qb]) * QB
                o0 = qb_off[qb]
                nc.sync.dma_start(
                    out=msk_sb[:, o0:o0 + w], in_=msk_d[:, o0:o0 + w]
                )

            qT = pp.tile([P, S], BF16, tag="qT")
            kT = pp.tile([P, S], BF16, tag="kT")
            v0 = pp.tile([P, NKT * VW], BF16, tag="v0")
            v1 = pp.tile([P, NKT * VW], BF16, tag="v1")
            nc.vector.memset(v0[:], 1.0)
            nc.vector.memset(v1[:], 1.0)
            on_sb = pp.tile([P, S], BF16, tag="on")

            # ---- Phase A: projections ----
            QC = 512
            with tc.tile_pool(name="pa", bufs=3, space="PSUM") as pa:
                for qc in range(NC4):          # kT = Wk^T X^T  (no bias)
                    ps = pa.tile([P, QC], F32, tag="qkps", name="kps")
                    for c in range(NC4):
                        nc.tensor.matmul(
                            ps[:],
                            lhsT=w_sb["k"][:, c * P:(c + 1) * P],
                            rhs=xt_sb[:, c * S + qc * QC: c * S + qc * QC + QC],
                            start=(c == 0),
                            stop=(c == NC4 - 1),
                        )
                    if qc % 2 == 0:
                        nc.scalar.copy(out=kT[:, qc * QC:(qc + 1) * QC], in_=ps[:])
                    else:
                        nc.vector.tensor_copy(
                            out=kT[:, qc * QC:(qc + 1) * QC], in_=ps[:]
                        )
                for qc in range(NC4):          # qT = Wq'^T X^T + bq'  (scaled)
                    ps = pa.tile([P, QC], F32, tag="qkps", name="qps")
                    for c in range(NC4):
                        nc.tensor.matmul(
                            ps[:],
                            lhsT=w_sb["q"][:, c * P:(c + 1) * P],
                            rhs=xt_sb[:, c * S + qc * QC: c * S + qc * QC + QC],
                            start=(c == 0),
                            stop=(c == NC4 - 1),
                        )
                    nc.vector.tensor_tensor(
                        out=qT[:, qc * QC:(qc + 1) * QC],
                        in0=ps[:],
                        in1=bq_sb[:].to_broadcast([P, QC]),
                        op=mybir.AluOpType.add,
                    )
                for kt in range(NKT):          # V direct form [tokens, dims]
                    vps = pa.tile([P, P], F32, tag="vps", name="vps")
                    for c in range(NC4):
                        nc.tensor.matmul(
                            vps[:],
                            lhsT=xt_sb[:, c * S + kt * P: c * S + kt * P + P],
                            rhs=w_sb["v"][:, c * P:(c + 1) * P],
                            start=(c == 0),
                            stop=(c == NC4 - 1),
                        )
                    d0 = kt * VW
                    if kt % 2 == 0:
                        nc.vector.tensor_copy(out=v0[:, d0:d0 + D], in_=vps[:, 0:D])
                        nc.scalar.copy(out=v1[:, d0:d0 + D], in_=vps[:, D:2 * D])
                    else:
                        nc.scalar.copy(out=v0[:, d0:d0 + D], in_=vps[:, 0:D])
                        nc.vector.tensor_copy(out=v1[:, d0:d0 + D], in_=vps[:, D:2 * D])

            # ---- Phase B: block-sparse attention + normalize + project ----
            with tc.tile_pool(name="ps_s", bufs=2, space="PSUM") as sp, \
                 tc.tile_pool(name="ps_o", bufs=2, space="PSUM") as opool, \
                 tc.tile_pool(name="ps_r", bufs=1, space="PSUM") as rp, \
                 tc.tile_pool(name="ps_pr", bufs=1, space="PSUM") as prp, \
                 tc.tile_pool(name="pb", bufs=3) as pb, \
                 tc.tile_pool(name="fin", bufs=3) as fsb:
                for qb in range(NQB):
                    kts = sched[qb]
                    nkt = len(kts)
                    qs = slice(qb * QB, (qb + 1) * QB)
                    o = opool.tile([P, 2 * QB], F32, tag="o", name="o")
                    groups = [kts[i:i + 2] for i in range(0, nkt, 2)]
                    pi = 0
                    for g in groups:
                        # s spans 2 banks: bank A (cols 0:2QB) holds h0
                        # scores for the group's pairs, bank B (2QB:4QB)
                        # holds h1.  Row-group-64 matmuls must not share a
                        # bank with row-group-0 ones (HW crash), so heads
                        # get separate banks; ident mask matmuls (full 128)
                        # may share with either.
                        ng = len(g)
                        s = sp.tile([P, 4 * QB], F32, tag="s", name="s")
                        pmt = pb.tile([P, 4 * QB], BF16, tag="pm", name="pm")
                        for jj, kt in enumerate(g):
                            a0 = jj * QB              # h0 region (bank A)
                            a1 = 2 * QB + jj * QB     # h1 region (bank B)
                            moff = qb_off[qb] + (pi + jj) * QB
                            if "norowtile" in DBG:
                                nc.tensor.matmul(
                                    s[:, a0:a0 + QB],
                                    lhsT=kT[:, kt * P:(kt + 1) * P],
                                    rhs=qT[:, qs],
                                    start=(jj == 0), stop=False,
                                )
                                nc.tensor.matmul(
                                    s[:, a1:a1 + QB],
                                    lhsT=kT[:, kt * P:(kt + 1) * P],
                                    rhs=qT[:, qs],
                                    start=(jj == 0), stop=False,
                                )
                            else:
                                nc.tensor.matmul(
                                    s[:, a0:a0 + QB],
                                    lhsT=kT[0:D, kt * P:(kt + 1) * P],
                                    rhs=qT[0:D, qs],
                                    start=(jj == 0), stop=False,
                                )
                                nc.tensor.matmul(
                                    s[:, a1:a1 + QB],
                                    lhsT=kT[D:P, kt * P:(kt + 1) * P],
                                    rhs=qT[D:P, qs],
                                    start=(jj == 0), stop=False,
                                )
                            nc.tensor.matmul(
                                s[:, a0:a0 + QB],
                                lhsT=idf8[:],
                                rhs=msk_sb[:, moff:moff + QB],
                                start=False, stop=(jj == ng - 1),
                            )
                            nc.tensor.matmul(
                                s[:, a1:a1 + QB],
                                lhsT=idf8[:],
                                rhs=msk_sb[:, moff:moff + QB],
                                start=False, stop=(jj == ng - 1),
                            )
                        if ng == 2:
                            nc.scalar.activation(
                                pmt[:], s[:], mybir.ActivationFunctionType.Exp
                            )
                        else:
                            nc.scalar.activation(
                                pmt[:, 0:QB], s[:, 0:QB],
                                mybir.ActivationFunctionType.Exp
                            )
                            nc.scalar.activation(
                                pmt[:, 2 * QB:3 * QB], s[:, 2 * QB:3 * QB],
                                mybir.ActivationFunctionType.Exp
                            )
                        for jj, kt in enumerate(g):
                            j = pi + jj
                            a0 = jj * QB
                            a1 = 2 * QB + jj * QB
                            nc.tensor.matmul(
                                o[0:VW, 0:QB],
                                lhsT=v0[:, kt * VW:(kt + 1) * VW],
                                rhs=pmt[:, a0:a0 + QB],
                                start=(j == 0), stop=False,
                            )
                            nc.tensor.matmul(
                                o[0:VW, QB:2 * QB],
                                lhsT=v1[:, kt * VW:(kt + 1) * VW],
                                rhs=pmt[:, a1:a1 + QB],
                                start=False, stop=(j == nkt - 1),
                            )
                        pi += len(g)
                        if "dump" in DBG and pi == len(g):
                            dt_ = pb.tile([P, 4 * QB], F32, tag="dmp", name="dmp")
                            nc.scalar.copy(out=dt_[:], in_=pmt[:])
                            nc.sync.dma_start(
                                out=dpm_d[:, qb * 4 * QB: (qb * 4 + 4) * QB],
                                in_=dt_[:])

                    if "dump" in DBG:
                        dt2 = pb.tile([P, 2 * QB], F32, tag="dmp2", name="dmp2")
                        nc.vector.tensor_copy(out=dt2[:], in_=o[:])
                        nc.sync.dma_start(
                            out=do_d[:, qb * 2 * QB:(qb + 1) * 2 * QB], in_=dt2[:])
                    if "nonorm" in DBG:
                        nc.vector.tensor_copy(out=on_sb[0:D, qs], in_=o[0:D, 0:QB])
                        nc.vector.tensor_copy(out=on_sb[D:P, qs], in_=o[0:D, QB:2 * QB])
                    else:
                        den_sb = pb.tile([1, 2 * QB], F32, tag="den", name="den_sb")
                        # custom-DVE recip misreads PSUM at partition base 64;
                        # stage the den row through SBUF first.
                        nc.scalar.copy(out=den_sb[:], in_=o[D:D + 1, :])
                        rd = pb.tile([1, 2 * QB], F32, tag="rd", name="rd")
                        nc.vector.reciprocal_approx_fast(out=rd[:], in_=den_sb[:])
                        rep = rp.tile([P, 2 * QB], F32, tag="rep", name="rep")
                        nc.tensor.matmul(
                            rep[:, 0:QB], lhsT=onesr[:], rhs=rd[0:1, 0:QB],
                            start=True, stop=False,
                        )
                        nc.tensor.matmul(
                            rep[:, QB:2 * QB], lhsT=onesr[:], rhs=rd[0:1, QB:2 * QB],
                            start=False, stop=True,
                        )
                        rep_sb = pb.tile([P, 2 * QB], BF16, tag="repsb", name="rep_sb")
                        nc.scalar.copy(out=rep_sb[:], in_=rep[:])
                        nc.vector.tensor_tensor(
                            out=on_sb[0:D, qs], in0=o[0:D, 0:QB],
                            in1=rep_sb[0:D, 0:QB],
                            op=mybir.AluOpType.mult,
                        )
                        nc.vector.tensor_tensor(
                            out=on_sb[D:P, qs], in0=o[0:D, QB:2 * QB],
                            in1=rep_sb[D:P, QB:2 * QB],
                            op=mybir.AluOpType.mult,
                        )
                    for t in range(QB // P):
                        qt = qb * (QB // P) + t
                        pr = prp.tile([P, DIM], F32, tag="pr", name="pr")
                        nc.tensor.matmul(
                            pr[:],
                            lhsT=on_sb[:, qt * P:(qt + 1) * P],
                            rhs=wo_sb[:],
                            start=True, stop=True,
                        )
                        ob = fsb.tile([P, DIM], BF16, tag="ob", name="ob")
                        if qt % 2 == 0:
                            nc.scalar.copy(out=ob[:], in_=pr[:])
                        else:
                            nc.vector.tensor_copy(out=ob[:], in_=pr[:])
                        nc.sync.dma_start(
                            out=out_d[qt * P:(qt + 1) * P, :], in_=ob[:]
                        )

                if "dump" in DBG:
                    for nm, src_t, dst in (("dq", qT, dq_d), ("dk", kT, dk_d),
                                           ("dv0", v0, dv0_d), ("dv1", v1, dv1_d),
                                           ("don", on_sb, don_d)):
                        w = src_t.shape[1]
                        for c0 in range(0, w, 2048):
                            cw = min(2048, w - c0)
                            dt3 = fsb.tile([P, 2048], F32, tag="dmp3", name="dmp3")
                            nc.vector.tensor_copy(out=dt3[:, 0:cw], in_=src_t[:, c0:c0 + cw])
                            nc.sync.dma_start(out=dst[:, c0:c0 + cw], in_=dt3[:, 0:cw])

    nc.compile()
    _CACHE[key] = nc
    return nc


def _pack(a):
    # [n*128, X] -> [128, n*X]
    n = a.shape[0] // P
    return np.ascontiguousarray(
        a.reshape(n, P, a.shape[1]).transpose(1, 0, 2).reshape(P, -1))


def make_in_maps(x, routes, w_qkv, b_qkv, w_out, order, rk, sched):
    x = np.asarray(x, np.float32)
    w_qkv = np.asarray(w_qkv, np.float32)
    b_qkv = np.asarray(b_qkv, np.float32)
    w_out = np.asarray(w_out, np.float32)

    Cm = np.zeros((S, S), bool)
    Cm[np.arange(S)[:, None], rk] = True          # permuted [q', k']
    cols = []
    for qb, kts in enumerate(sched):
        blk = Cm[qb * QB:(qb + 1) * QB]
        for kt in kts:
            sub = blk[:, kt * P:(kt + 1) * P]     # [256 q, 128 k]
            cols.append(np.where(sub.T, 0.0, MASKNEG))
    msk = np.concatenate(cols, axis=1).astype(NPF8)

    xt = [
        _pack(np.ascontiguousarray(x[b].T[:, order])).astype(NPBF16)
        for b in range(B)
    ]

    in_maps = []
    for core in range(8):
        b = core // 4
        hp = core % 4
        col = hp * P
        wq = _pack(w_qkv[:, col:col + P] * SCALE).astype(NPBF16)
        wk = _pack(w_qkv[:, DIM + col:DIM + col + P]).astype(NPBF16)
        wv = _pack(w_qkv[:, 2 * DIM + col:2 * DIM + col + P]).astype(NPBF16)
        bq = (b_qkv[col:col + P] * SCALE).astype(np.float32).reshape(P, 1)
        wo = np.ascontiguousarray(w_out[col:col + P, :]).astype(NPBF16)
        in_maps.append(dict(
            xt=xt[b], wq=wq, wk=wk, wv=wv, bq=bq, msk=msk, wo=wo,
        ))
    return in_maps


def run(inputs, trace=False, trace_cores=None):
    routes = np.asarray(inputs["routes"])
    order, rk, sched = _schedule(routes)
    nc = build_nc(sched)
    in_maps = make_in_maps(
        inputs["x"], routes, inputs["w_qkv"], inputs["b_qkv"],
        inputs["w_out"], order, rk, sched,
    )
    res = run_bass_kernel_spmd(
        nc, in_maps, list(range(8)), trace=trace, trace_cores=trace_cores,
    )
    b_qkv = np.asarray(inputs["b_qkv"], np.float32)
    w_out = np.asarray(inputs["w_out"], np.float32)
    bias = np.asarray(inputs["b_out"], np.float32) + b_qkv[2 * DIM:] @ w_out
    final = np.zeros((B, S, DIM), np.float32)
    for core in range(8):
        final[core // 4][order] += np.asarray(res.results[core]["out"], np.float32)
    final += bias[None, None, :]
    return final, res


def kernel(**inputs):
    final, _ = run(inputs, trace=False)
    return final
